# revision 31
# baseline (speedup 1.0000x reference)
# Triplet-margin loss kernel for Trainium2 (Bass/Tile), batch-sharded
# across 8 NeuronCores.
#
# reference math (torch F.pairwise_distance semantics):
#   d_ap[b,p] = || anc[b] - pos[b,p] + eps ||_2   (eps=1e-6, negligible)
#   d_an[b,n] = || anc[b] - neg[b,n] + eps ||_2
#   loss = mean_{b,p,n} max(d_ap[b,p] - d_an[b,n] + margin, 0)
#
# Strategy (v5): the whole per-core input (26.2MB fp32) is cast to bf16
# during DMA (SWDGE cast) and kept fully resident in SBUF (13.1MB), with
# all load DMAs issued upfront in consumption order so the DMA engines
# stream HBM at line rate start-to-finish (~410GB/s read observed).
# GpSimd does nothing but issue the cast DMAs; anc rides the parallel
# HWDGE (sync) queue as fp32 and is cast+replicated on DVE. Each z-slice
# computes d^2 = sum((x - a)^2) one of two ways, split to balance engine
# busy time under the DMA roof:
#   ACT slices: batched DVE subtract u = x - [a|a|..] (bf16 tensor_tensor
#     at 2x mode) + ACT activation Square with accumulate;
#   DVE slices: one fused custom-DVE op SQDIFF_REDUCE_ANT
#     (out=(x-a)^2, accum_out=sum) straight from the chunk tile.
# Per tile the chunk order is neg-first / pos-last with the final pos
# chunks kept small, and sqrt + the (p,n) combination are split per pos
# group so almost all of the combine runs before the last byte lands.
# The (p,n) combination uses scalar_tensor_tensor(subtract, min 0,
# accum) which yields -sum_n relu(d_ap - d_an + margin) per (b,p) in
# one op. Each core returns per-partition partial sums [128, 2]; the
# host sums and scales.

from operator import add as _op_add

import numpy as np

import concourse.bacc as bacc
import concourse.mybir as mybir
import concourse.tile as tile
from concourse import bass_utils
from concourse import dve_ops
from concourse.dve_spec import Spec, Src0, Src1, sq as _sq, lower as _dve_lower
from concourse.dve_spec import _has_src1
from concourse.dve_uop import DveOpSpec
from concourse.dve_table_gen import dve_ver_for

B, Z = 2048, 1024
NUM_POS, NUM_NEG = 8, 16
NJ = NUM_POS + NUM_NEG
MARGIN = 1.0
N_CORES = 8
BL = B // N_CORES  # 256 rows of anc per core
P = 128
NT = BL // P  # 2 batch-tiles per core

# Per-tile chunk schedule: (kind, first_slice, n_slices, n_act) —
# the first n_act slices of the chunk square on ACT (via the batched
# subtract), the rest go through the fused DVE op. neg slices are
# jj 8..23, pos are jj 0..7; negs are issued first so the pos-dependent
# combine tail is tiny, and the first chunk is small so compute starts
# as early as possible.
CHUNKS = [
    ("neg", 8, 2, 1),       # jj 8,9
    ("neg", 10, 4, 3),      # jj 10..13
    ("neg", 14, 4, 3),      # jj 14..17
    ("neg", 18, 4, 3),      # jj 18..21
    ("neg", 22, 2, 1),      # jj 22,23
    ("pos", 0, 4, 3),       # jj 0..3
    ("pos", 4, 2, 1),       # jj 4,5
    ("pos", 6, 2, 1),       # jj 6,7
]

F32 = mybir.dt.float32
BF16 = mybir.dt.bfloat16
AF = mybir.ActivationFunctionType
OP = mybir.AluOpType


# ---- custom DVE op: out = (in0 - in1)^2, accum_out = sum(out) ----
def _ref_sqdiff(in0, in1, s0, s1, imm2):
    b = ((in0.astype(np.float32) - in1.astype(np.float32)) ** 2).astype(np.float32)
    return b, b.reshape(b.shape[0], -1).sum(axis=-1, keepdims=True)


def _register_sqdiff():
    name = "SQDIFF_REDUCE_ANT"
    for op in dve_ops.OPS:
        if op.name == name:
            return op
    spec = Spec(body=_sq(Src0 - Src1), accum=_op_add, reference=_ref_sqdiff)
    ver = dve_ver_for("TRN2")
    sha = DveOpSpec(
        name=name, opcode=None, uops=_dve_lower(spec, ver=ver), rd1_en=_has_src1(spec)
    ).sha(ver)
    op = dve_ops.DveOp(name, spec, subdim=False, uops_sha={ver: sha})
    dve_ops.OPS.append(op)
    dve_ops.CUSTOM_DVE_SPECS[name] = spec
    dve_ops._SUB_OPCODE_FOR_NAME[name] = dve_ops._CUSTOM_DVE_ROW_BASE + len(dve_ops.OPS) - 1
    return op


SQDIFF = _register_sqdiff()


def _emit(tc, nc, anc, pos, neg, out):
    v = nc.vector
    act = nc.scalar
    gp = nc.gpsimd
    pos2 = pos.rearrange("(b j) z -> b (j z)", j=NUM_POS)  # [BL, 8*Z]
    neg2 = neg.rearrange("(b j) z -> b (j z)", j=NUM_NEG)  # [BL, 16*Z]
    with (
        tc.tile_pool(name="xp", bufs=1) as xp,
        tc.tile_pool(name="up1", bufs=3) as up1,
        tc.tile_pool(name="up3", bufs=3) as up3,
        tc.tile_pool(name="scp", bufs=1) as scp,
        tc.tile_pool(name="smp", bufs=2) as smp,
        tc.tile_pool(name="opool", bufs=1) as opool,
    ):
        osb = opool.tile([P, NT], F32, name="osb")
        dve_scr = scp.tile([P, Z], BF16, name="dve_scr")
        act_scr = scp.tile([P, Z], BF16, name="act_scr")
        ts_out = scp.tile([P, NUM_NEG], F32, name="ts_out")
        zero_n = opool.tile([P, NUM_NEG], F32, name="zero_n")
        v.memset(zero_n[:, :], 0.0)

        # ---- Phase 0: issue every load upfront, in consumption order ----
        # anc rides the HWDGE (sync) queue in parallel with the SWDGE
        # chunk stream; cast fp32->bf16 happens in the DVE copy below.
        ancf = []
        aaaa = []
        for t in range(NT):
            af = xp.tile([P, Z], F32, name=f"af_{t}")
            nc.sync.dma_start(af[:, :], anc[t * P : (t + 1) * P, :])
            ancf.append(af)
            aaaa.append(xp.tile([P, 4 * Z], BF16, name=f"a4_{t}"))
        chunks = {}
        for t in range(NT):
            b0 = t * P
            for ci, (kind, j0, nsl, _) in enumerate(CHUNKS):
                w = nsl * Z
                xt = xp.tile([P, w], BF16, name=f"x_{t}_{ci}")
                if kind == "pos":
                    src = pos2[b0 : b0 + P, j0 * Z : j0 * Z + w]
                else:
                    src = neg2[b0 : b0 + P, (j0 - NUM_POS) * Z : (j0 - NUM_POS) * Z + w]
                gp.dma_start(xt[:, :], src)
                chunks[(t, ci)] = xt

        # ---- Phase 1: per-tile distance slices + split combine ----
        for t in range(NT):
            a4 = aaaa[t]
            # cast anc to bf16 and replicate into all 4 z-slice positions
            v.tensor_copy(a4[:, 0:Z], ancf[t][:, :])
            v.tensor_copy(a4[:, Z : 2 * Z], a4[:, 0:Z])
            v.tensor_copy(a4[:, 2 * Z : 4 * Z], a4[:, 0 : 2 * Z])
            d2 = smp.tile([P, NJ], F32, name="d2")
            dt_ = smp.tile([P, NJ], F32, name="dt_")
            s_m = smp.tile([P, NUM_POS], F32, name="s_m")
            lp = smp.tile([P, NUM_POS], F32, name="lp")
            for ci, (kind, j0, nsl, n_act) in enumerate(CHUNKS):
                xt = chunks[(t, ci)]
                # batched subtract for the ACT-owned leading slices
                wa = n_act * Z
                pool = up3 if n_act >= 2 else up1
                ut = pool.tile([P, wa], BF16, name=f"u{n_act}")
                v.tensor_tensor(
                    out=ut[:, :], in0=xt[:, 0:wa], in1=a4[:, 0:wa], op=OP.subtract
                )
                for q in range(n_act):
                    jj = j0 + q
                    act.activation(
                        act_scr[:, :],
                        ut[:, q * Z : (q + 1) * Z],
                        AF.Square,
                        accum_out=d2[:, jj : jj + 1],
                    )
                for q in range(n_act, nsl):
                    jj = j0 + q
                    v._custom_dve(
                        SQDIFF,
                        out=dve_scr[:, :],
                        in0=xt[:, q * Z : (q + 1) * Z],
                        in1=a4[:, q * Z : (q + 1) * Z],
                        accum_out=d2[:, jj : jj + 1],
                    )
                if kind == "neg" and j0 + nsl == NJ:
                    # all negs done: d_an = sqrt(d2_neg)
                    act.activation(dt_[:, NUM_POS:NJ], d2[:, NUM_POS:NJ], AF.Sqrt)
                elif kind == "pos":
                    # this pos group done: d_ap group, s = d_ap + margin,
                    # then lp[:,p] = sum_n min(d_an - s_p, 0)
                    act.activation(
                        dt_[:, j0 : j0 + nsl], d2[:, j0 : j0 + nsl], AF.Sqrt
                    )
                    v.tensor_scalar_add(
                        s_m[:, j0 : j0 + nsl], dt_[:, j0 : j0 + nsl], MARGIN
                    )
                    for p_i in range(j0, j0 + nsl):
                        v.scalar_tensor_tensor(
                            out=ts_out[:, :],
                            in0=dt_[:, NUM_POS:NJ],
                            scalar=s_m[:, p_i : p_i + 1],
                            in1=zero_n[:, :],
                            op0=OP.subtract,
                            op1=OP.min,
                            accum_out=lp[:, p_i : p_i + 1],
                        )
            v.reduce_sum(osb[:, t : t + 1], lp[:, :], axis=mybir.AxisListType.X)
        nc.sync.dma_start(out[:, :], osb[:, :])


_NC_CACHE = None


def build():
    global _NC_CACHE
    if _NC_CACHE is None:
        nc = bacc.Bacc(
            "TRN2", target_bir_lowering=False, debug=False, num_devices=N_CORES
        )
        anc = nc.dram_tensor("anc", (BL, Z), F32, kind="ExternalInput").ap()
        pos = nc.dram_tensor("pos", (BL * NUM_POS, Z), F32, kind="ExternalInput").ap()
        neg = nc.dram_tensor("neg", (BL * NUM_NEG, Z), F32, kind="ExternalInput").ap()
        out = nc.dram_tensor("out", (P, NT), F32, kind="ExternalOutput").ap()
        with tile.TileContext(nc) as tc:
            _emit(tc, nc, anc, pos, neg, out)
        nc.compile()
        _NC_CACHE = nc
    return _NC_CACHE


def make_in_maps(anc_embedding, pos_embedding, neg_embedding):
    anc_embedding = np.asarray(anc_embedding, dtype=np.float32)
    pos_embedding = np.asarray(pos_embedding, dtype=np.float32)
    neg_embedding = np.asarray(neg_embedding, dtype=np.float32)
    in_maps = []
    for c in range(N_CORES):
        in_maps.append(
            {
                "anc": np.ascontiguousarray(anc_embedding[c * BL : (c + 1) * BL]),
                "pos": np.ascontiguousarray(
                    pos_embedding[c * BL * NUM_POS : (c + 1) * BL * NUM_POS]
                ),
                "neg": np.ascontiguousarray(
                    neg_embedding[c * BL * NUM_NEG : (c + 1) * BL * NUM_NEG]
                ),
            }
        )
    return in_maps


def combine(outs):
    # outs: list of [P, NT] per-core partial sums of min(d_an - s, 0)
    total = sum(o.astype(np.float64).sum() for o in outs)
    return np.float32(-total / (B * NUM_POS * NUM_NEG))


def kernel(anc_embedding, pos_embedding, neg_embedding):
    nc = build()
    in_maps = make_in_maps(anc_embedding, pos_embedding, neg_embedding)
    res = bass_utils.run_bass_kernel_spmd(nc, in_maps, core_ids=list(range(N_CORES)))
    return combine([r["out"] for r in res.results])


# revision 34
# speedup vs baseline: 1.0508x; 1.0508x over previous
# Triplet-margin loss kernel for Trainium2 (Bass/Tile), batch-sharded
# across 8 NeuronCores.
#
# reference math (torch F.pairwise_distance semantics):
#   d_ap[b,p] = || anc[b] - pos[b,p] + eps ||_2   (eps=1e-6, negligible)
#   d_an[b,n] = || anc[b] - neg[b,n] + eps ||_2
#   loss = mean_{b,p,n} max(d_ap[b,p] - d_an[b,n] + margin, 0)
#
# Strategy (v5): the whole per-core input (26.2MB fp32) is cast to bf16
# during DMA (SWDGE cast) and kept fully resident in SBUF (13.1MB), with
# all load DMAs issued upfront in consumption order so the DMA engines
# stream HBM at line rate start-to-finish (~410GB/s read observed).
# GpSimd does nothing but issue the cast DMAs; anc rides the parallel
# HWDGE (sync) queue as fp32 and is cast+replicated on DVE. Each z-slice
# computes d^2 = sum((x - a)^2) one of two ways, split to balance engine
# busy time under the DMA roof:
#   ACT slices: batched DVE subtract u = x - [a|a|..] (bf16 tensor_tensor
#     at 2x mode) + ACT activation Square with accumulate;
#   DVE slices: one fused custom-DVE op SQDIFF_REDUCE_ANT
#     (out=(x-a)^2, accum_out=sum) straight from the chunk tile.
# Per tile the chunk order is neg-first / pos-last with the final pos
# chunks kept small, and sqrt + the (p,n) combination are split per pos
# group so almost all of the combine runs before the last byte lands.
# The (p,n) combination uses scalar_tensor_tensor(subtract, min 0,
# accum) which yields -sum_n relu(d_ap - d_an + margin) per (b,p) in
# one op. Each core returns per-partition partial sums [128, 2]; the
# host sums and scales.

from operator import add as _op_add

import numpy as np

import concourse.bacc as bacc
import concourse.mybir as mybir
import concourse.tile as tile
from concourse import bass_utils
from concourse import dve_ops
from concourse.dve_spec import Spec, Src0, Src1, sq as _sq, lower as _dve_lower
from concourse.dve_spec import _has_src1
from concourse.dve_uop import DveOpSpec
from concourse.dve_table_gen import dve_ver_for

B, Z = 2048, 1024
NUM_POS, NUM_NEG = 8, 16
NJ = NUM_POS + NUM_NEG
MARGIN = 1.0
N_CORES = 8
BL = B // N_CORES  # 256 rows of anc per core
P = 128
NT = BL // P  # 2 batch-tiles per core

# Per-tile chunk schedule: (kind, first_slice, n_slices, n_act) —
# the first n_act slices of the chunk square on ACT (via the batched
# subtract), the rest go through the fused DVE op. neg slices are
# jj 8..23, pos are jj 0..7; negs are issued first so the pos-dependent
# combine tail is tiny, and the first chunk is small so compute starts
# as early as possible.
CHUNKS = [
    ("neg", 8, 2, 1),       # jj 8,9
    ("neg", 10, 4, 3),      # jj 10..13
    ("neg", 14, 4, 3),      # jj 14..17
    ("neg", 18, 4, 3),      # jj 18..21
    ("neg", 22, 2, 1),      # jj 22,23
    ("pos", 0, 4, 3),       # jj 0..3
    ("pos", 4, 2, 1),       # jj 4,5
    ("pos", 6, 2, 1),       # jj 6,7
]

F32 = mybir.dt.float32
BF16 = mybir.dt.bfloat16
AF = mybir.ActivationFunctionType
OP = mybir.AluOpType


# ---- custom DVE op: out = (in0 - in1)^2, accum_out = sum(out) ----
def _ref_sqdiff(in0, in1, s0, s1, imm2):
    b = ((in0.astype(np.float32) - in1.astype(np.float32)) ** 2).astype(np.float32)
    return b, b.reshape(b.shape[0], -1).sum(axis=-1, keepdims=True)


def _register_sqdiff():
    name = "SQDIFF_REDUCE_ANT"
    for op in dve_ops.OPS:
        if op.name == name:
            return op
    spec = Spec(body=_sq(Src0 - Src1), accum=_op_add, reference=_ref_sqdiff)
    ver = dve_ver_for("TRN2")
    sha = DveOpSpec(
        name=name, opcode=None, uops=_dve_lower(spec, ver=ver), rd1_en=_has_src1(spec)
    ).sha(ver)
    op = dve_ops.DveOp(name, spec, subdim=False, uops_sha={ver: sha})
    dve_ops.OPS.append(op)
    dve_ops.CUSTOM_DVE_SPECS[name] = spec
    dve_ops._SUB_OPCODE_FOR_NAME[name] = dve_ops._CUSTOM_DVE_ROW_BASE + len(dve_ops.OPS) - 1
    return op


SQDIFF = _register_sqdiff()


def _emit(tc, nc, anc, pos, neg, out):
    v = nc.vector
    act = nc.scalar
    gp = nc.gpsimd
    pos2 = pos.rearrange("(b j) z -> b (j z)", j=NUM_POS)  # [BL, 8*Z]
    neg2 = neg.rearrange("(b j) z -> b (j z)", j=NUM_NEG)  # [BL, 16*Z]
    with (
        tc.tile_pool(name="xp", bufs=1) as xp,
        tc.tile_pool(name="up1", bufs=3) as up1,
        tc.tile_pool(name="up3", bufs=3) as up3,
        tc.tile_pool(name="scp", bufs=1) as scp,
        tc.tile_pool(name="smp", bufs=2) as smp,
        tc.tile_pool(name="opool", bufs=1) as opool,
    ):
        osb = opool.tile([P, NT], F32, name="osb")
        dve_scr = scp.tile([P, Z], BF16, name="dve_scr")
        act_scr = scp.tile([P, Z], BF16, name="act_scr")
        ts_out = scp.tile([P, NUM_NEG], F32, name="ts_out")
        zero_n = opool.tile([P, NUM_NEG], F32, name="zero_n")
        v.memset(zero_n[:, :], 0.0)

        # ---- Phase 0: issue every load upfront, in consumption order ----
        # anc rides the HWDGE (sync) queue in parallel with the SWDGE
        # chunk stream; cast fp32->bf16 happens in the DVE copy below.
        ancf = []
        aaaa = []
        for t in range(NT):
            af = xp.tile([P, Z], F32, name=f"af_{t}")
            nc.sync.dma_start(af[:, :], anc[t * P : (t + 1) * P, :])
            ancf.append(af)
            aaaa.append(xp.tile([P, 4 * Z], BF16, name=f"a4_{t}"))
        chunks = {}
        for t in range(NT):
            b0 = t * P
            for ci, (kind, j0, nsl, _) in enumerate(CHUNKS):
                w = nsl * Z
                xt = xp.tile([P, w], BF16, name=f"x_{t}_{ci}")
                if kind == "pos":
                    src = pos2[b0 : b0 + P, j0 * Z : j0 * Z + w]
                else:
                    src = neg2[b0 : b0 + P, (j0 - NUM_POS) * Z : (j0 - NUM_POS) * Z + w]
                gp.dma_start(xt[:, :], src)
                chunks[(t, ci)] = xt

        # ---- Phase 1: per-tile distance slices + split combine ----
        for t in range(NT):
            a4 = aaaa[t]
            # cast anc to bf16 and replicate into all 4 z-slice positions
            v.tensor_copy(a4[:, 0:Z], ancf[t][:, :])
            v.tensor_copy(a4[:, Z : 2 * Z], a4[:, 0:Z])
            v.tensor_copy(a4[:, 2 * Z : 4 * Z], a4[:, 0 : 2 * Z])
            d2 = smp.tile([P, NJ], F32, name="d2")
            dt_ = smp.tile([P, NJ], F32, name="dt_")
            s_m = smp.tile([P, NUM_POS], F32, name="s_m")
            lp = smp.tile([P, NUM_POS], F32, name="lp")
            for ci, (kind, j0, nsl, n_act) in enumerate(CHUNKS):
                xt = chunks[(t, ci)]
                # batched subtract for the ACT-owned leading slices
                wa = n_act * Z
                pool = up3 if n_act >= 2 else up1
                ut = pool.tile([P, wa], BF16, name=f"u{n_act}")
                v.tensor_tensor(
                    out=ut[:, :], in0=xt[:, 0:wa], in1=a4[:, 0:wa], op=OP.subtract
                )
                for q in range(n_act):
                    jj = j0 + q
                    act.activation(
                        act_scr[:, :],
                        ut[:, q * Z : (q + 1) * Z],
                        AF.Square,
                        accum_out=d2[:, jj : jj + 1],
                    )
                for q in range(n_act, nsl):
                    jj = j0 + q
                    v._custom_dve(
                        SQDIFF,
                        out=dve_scr[:, :],
                        in0=xt[:, q * Z : (q + 1) * Z],
                        in1=a4[:, q * Z : (q + 1) * Z],
                        accum_out=d2[:, jj : jj + 1],
                    )
                if kind == "neg" and j0 + nsl == NJ:
                    # all negs done: d_an = sqrt(d2_neg)
                    act.activation(dt_[:, NUM_POS:NJ], d2[:, NUM_POS:NJ], AF.Sqrt)
                elif kind == "pos":
                    # this pos group done: d_ap group, s = d_ap + margin,
                    # then lp[:,p] = sum_n min(d_an - s_p, 0)
                    act.activation(
                        dt_[:, j0 : j0 + nsl], d2[:, j0 : j0 + nsl], AF.Sqrt
                    )
                    v.tensor_scalar_add(
                        s_m[:, j0 : j0 + nsl], dt_[:, j0 : j0 + nsl], MARGIN
                    )
                    for p_i in range(j0, j0 + nsl):
                        v.scalar_tensor_tensor(
                            out=ts_out[:, :],
                            in0=dt_[:, NUM_POS:NJ],
                            scalar=s_m[:, p_i : p_i + 1],
                            in1=zero_n[:, :],
                            op0=OP.subtract,
                            op1=OP.min,
                            accum_out=lp[:, p_i : p_i + 1],
                        )
            v.reduce_sum(osb[:, t : t + 1], lp[:, :], axis=mybir.AxisListType.X)
        nc.sync.dma_start(out[:, :], osb[:, :])


_NC_CACHE = None


def build():
    global _NC_CACHE
    if _NC_CACHE is None:
        nc = bacc.Bacc(
            "TRN2", target_bir_lowering=False, debug=False, num_devices=N_CORES
        )
        anc = nc.dram_tensor("anc", (BL, Z), F32, kind="ExternalInput").ap()
        pos = nc.dram_tensor("pos", (BL * NUM_POS, Z), F32, kind="ExternalInput").ap()
        neg = nc.dram_tensor("neg", (BL * NUM_NEG, Z), F32, kind="ExternalInput").ap()
        out = nc.dram_tensor("out", (P, NT), F32, kind="ExternalOutput").ap()
        with tile.TileContext(nc) as tc:
            _emit(tc, nc, anc, pos, neg, out)
        nc.compile()
        _NC_CACHE = nc
    return _NC_CACHE


def make_in_maps(anc_embedding, pos_embedding, neg_embedding):
    anc_embedding = np.asarray(anc_embedding, dtype=np.float32)
    pos_embedding = np.asarray(pos_embedding, dtype=np.float32)
    neg_embedding = np.asarray(neg_embedding, dtype=np.float32)
    in_maps = []
    for c in range(N_CORES):
        in_maps.append(
            {
                "anc": np.ascontiguousarray(anc_embedding[c * BL : (c + 1) * BL]),
                "pos": np.ascontiguousarray(
                    pos_embedding[c * BL * NUM_POS : (c + 1) * BL * NUM_POS]
                ),
                "neg": np.ascontiguousarray(
                    neg_embedding[c * BL * NUM_NEG : (c + 1) * BL * NUM_NEG]
                ),
            }
        )
    return in_maps


def combine(outs):
    # outs: list of [P, NT] per-core partial sums of min(d_an - s, 0)
    total = sum(o.astype(np.float64).sum() for o in outs)
    return np.float32(-total / (B * NUM_POS * NUM_NEG))


def kernel(anc_embedding, pos_embedding, neg_embedding):
    nc = build()
    in_maps = make_in_maps(anc_embedding, pos_embedding, neg_embedding)
    res = bass_utils.run_bass_kernel_spmd(nc, in_maps, core_ids=list(range(N_CORES)))
    return combine([r["out"] for r in res.results])
